# revision 22
# baseline (speedup 1.0000x reference)
"""Trainium2 Bass kernel for nn_PolicyHead_1Trunk (scatter_memory).

Computation (reference):
    h = x @ lin_w.T + lin_b                  # [N, 256]
    h = batchnorm(h) (training stats over N) ; relu
    v = (h @ fin_w.T + fin_b)[:, 0]          # [N]
    out = scatter_add(v, batch) -> [256, 4096]; log_softmax rows

Strategy:
  * batch is the identity COO pattern [i // 2048, i % 2048] (verified on
    host; falls back to a numpy path if not).
  * BN batch statistics are folded into a per-channel affine on the host.
    To make fp8 quantization error *systematically* cancel, the affine is
    computed from the statistics of the QUANTIZED computation (x8^T x8 with
    the quantized weights, scale-refinement iterations), so the device's
    h has exactly the right mean and (to ~0.1%) the right variance; only
    per-node quantization noise survives, which the 2e-2 gate forgives.
  * x is shipped as fp8 e4m3 in DoubleRow interleave [128, 2, NLOC]
    (ki, ks, n) = x[n, ks*128+ki]: 16 MB/core of HBM traffic instead of
    64 MB.  The main matmul is a single K=256 DoubleRow fp8 matmul per
    512-node slice (2x PE throughput vs bf16).
  * Data-parallel over graphs: core i owns rows [i*65536, (i+1)*65536)
    (32 whole graphs).
  * Per core: stream x8 super-batches (6 graphs = 3 MB per DMA), DoubleRow
    matmul with the BN-folded fp8 weights (mh-outer so the stationary only
    changes twice per super-batch), fused bias+relu (ACT/DVE 13:11 split)
    into fp8 rt tiles, then a plain-fp8 fin matvec (runs at bf16 rate) via
    masked stationaries into a persistent PSUM tile [96, 2048] — graph g
    lives at partition 32*(g%3) + g//3, so fin matvecs of 3 consecutive
    graphs run concurrently on PE column-groups 0/32/64 — and finally a
    shift-free log-softmax epilogue over [32, 4096] rows including the
    2048 implicit zero entries (v is O(1) so exp cannot overflow).
"""

import os
import sys

import numpy as np

for _p in ("/opt/trn_rl_repo", "/root/.axon_site/_ro/trn_rl_repo"):
    if os.path.isdir(_p) and _p not in sys.path:
        sys.path.insert(0, _p)

C = 256           # channels
NPG = 2048        # nodes per graph
NG = 256          # graphs
N = NG * NPG      # 524288 nodes
AS = 4096         # action size
NCORES = 8
GPC = NG // NCORES          # 32 graphs per core
NLOC = GPC * NPG            # 65536 rows per core
BN_EPS = 1e-5

_PROG = None      # cached — compile once per process
TRACE = False     # test.py can flip this for ntff profiling
LAST_RESULTS = None


def _build_program():
    import concourse.bass as bass
    import concourse.tile as tile
    from concourse import bacc, mybir
    from contextlib import ExitStack

    f32 = mybir.dt.float32
    f8 = mybir.dt.float8e4
    AF = mybir.ActivationFunctionType
    ALU = mybir.AluOpType
    AX = mybir.AxisListType
    DR = mybir.MatmulPerfMode.DoubleRow

    nc = bacc.Bacc(
        "TRN2", target_bir_lowering=False, debug=False, enable_asserts=False
    )

    # x in DoubleRow interleave: x8[ki, ks, n] = x[n, ks*128+ki] (fp8 e4m3)
    x8 = nc.dram_tensor("x8", [128, 2, NLOC], f8, kind="ExternalInput").ap()
    # weights in DoubleRow interleave: wt8[ki, ks, m] = W_dev[ks*128+ki, m]
    wt8 = nc.dram_tensor("wt8", [128, 2, C], f8, kind="ExternalInput").ap()
    # fwm8[k, kh, g*32 + j] = fin_w[kh*128+k] * (j == g//3): masked copies
    # of the final projection.  An M=32 matmul with this stationary writes
    # v into one PSUM partition and +0 elsewhere, so PSUM accumulation
    # doubles as the per-graph scatter.
    fwm8 = nc.dram_tensor("fwm8", [128, 2, GPC * GPC], f8,
                          kind="ExternalInput").ap()
    bv = nc.dram_tensor("bv", [C, 1], f32, kind="ExternalInput").ap()
    fb = nc.dram_tensor("fb", [96, 1], f32, kind="ExternalInput").ap()
    out_d = nc.dram_tensor("out", [96, AS], f32, kind="ExternalOutput").ap()

    CHUNK = NPG           # 2048 rows per chunk == one graph
    NCH = NLOC // CHUNK   # 32 chunks
    SB = 6                # graphs per super-batch (one x DMA, 2 LDWEIGHTS)
    SUB = 1024            # columns per PSUM tile
    MM = 512              # output columns per matmul

    with tile.TileContext(nc) as tc, ExitStack() as ctx:
        consts = ctx.enter_context(tc.tile_pool(name="consts", bufs=1))
        xpool = ctx.enter_context(tc.tile_pool(name="x", bufs=3))
        rpool = ctx.enter_context(tc.tile_pool(name="relu", bufs=2))
        hpool = ctx.enter_context(tc.tile_pool(name="h", bufs=2, space="PSUM"))
        vpool = ctx.enter_context(tc.tile_pool(name="v", bufs=1, space="PSUM"))
        epool = ctx.enter_context(tc.tile_pool(name="epi", bufs=1))

        # ---- constants into SBUF via the scalar HWDGE queue (fast setup;
        # the sync queue is kept free to start streaming x immediately) ----
        wt8_sb = consts.tile([128, 2, C], f8, tag="wt8")
        nc.scalar.dma_start(wt8_sb[:], wt8[:, :, :])
        bv_sb = []   # [128, 1] per output-channel half
        for mh in range(2):
            t = consts.tile([128, 1], f32, tag=f"bv{mh}")
            nc.scalar.dma_start(t[:], bv[mh * 128:(mh + 1) * 128, :])
            bv_sb.append(t)
        fwm8_sb = consts.tile([128, 2, GPC * GPC], f8, tag="fwm8")
        nc.scalar.dma_start(fwm8_sb[:], fwm8[:, :, :])
        fb_sb = consts.tile([96, 1], f32, tag="fb")
        nc.scalar.dma_start(fb_sb[:], fb[:, :])

        # warm the ACT Relu table so the first bias+relu doesn't stall the
        # PE->relu pipeline behind a table load
        warm = epool.tile([1, 2], f32, tag="warm")
        nc.scalar.activation(warm[0:1, 0:1], fb_sb[0:1, 0:1], AF.Relu)

        # persistent PSUM accumulator for v: graph g lives at partition
        # 32*(g%3) + g//3 — g%3 selects the PE column-group so that the
        # fin matvecs of 3 consecutive graphs run concurrently on
        # column-groups 0/32/64 of the systolic array.
        vps = vpool.tile([96, CHUNK], f32, tag="vps")
        NSB = (NCH + SB - 1) // SB              # 6 super-batches
        last_bb = [(NCH - 1 - ci) // 3 for ci in range(3)]
        pending_fin = []

        for k in range(NSB):
            graphs = [g for g in range(k * SB, min((k + 1) * SB, NCH))]
            ng = len(graphs)
            # one DMA for the whole super-batch's x (6 graphs = 3 MB);
            # super-batch 0 is split per-graph so the first matmul can
            # start sooner
            xb = xpool.tile([128, 2, SB * CHUNK], f8, tag="xb")
            c0 = graphs[0] * CHUNK
            if k == 0:
                nc.sync.dma_start(
                    xb[:, :, 0:CHUNK // 2], x8[:, :, 0:CHUNK // 2]
                )
                nc.sync.dma_start(
                    xb[:, :, CHUNK // 2:CHUNK], x8[:, :, CHUNK // 2:CHUNK]
                )
                for ci in range(1, ng):
                    nc.sync.dma_start(
                        xb[:, :, ci * CHUNK:(ci + 1) * CHUNK],
                        x8[:, :, c0 + ci * CHUNK:c0 + (ci + 1) * CHUNK],
                    )
            else:
                nc.sync.dma_start(
                    xb[:, :, 0:ng * CHUNK], x8[:, :, c0:c0 + ng * CHUNK]
                )
            # rt tiles: [128, 2, SUB] fp8, mh halves stacked — one tile per
            # (ci, s); the fin matvec reads each half as a [128, 512] slice
            rts = [[rpool.tile([128, 2, SUB], f8, tag=f"r{ci}_{s}",
                               name=f"rt{ci}{s}")
                    for s in range(CHUNK // SUB)] for ci in range(ng)]
            ridx = [0]

            def relu_ops(mh, cis):
                for ci in cis:
                    for s in range(CHUNK // SUB):
                        hps = hpool.tile([128, SUB], f32, tag="hps",
                                         name="hps")
                        for ns in range(SUB // MM):
                            col = ci * CHUNK + s * SUB + ns * MM
                            nc.tensor.matmul(
                                hps[:, ns * MM:(ns + 1) * MM],
                                lhsT=wt8_sb[:, :, mh * 128:(mh + 1) * 128],
                                rhs=xb[:, :, col:col + MM],
                                start=True,
                                stop=True,
                                perf_mode=DR,
                            )
                        rt = rts[ci][s][:, mh, :]
                        if ridx[0] % 2 == 0 or (ridx[0] == 23 and k % 2 == 0):
                            nc.scalar.activation(
                                rt, hps[:], AF.Relu,
                                bias=bv_sb[mh][:, 0:1],
                            )
                        else:
                            nc.vector.tensor_scalar(
                                out=rt, in0=hps[:],
                                scalar1=bv_sb[mh][:, 0:1], scalar2=0.0,
                                op0=ALU.add, op1=ALU.max,
                            )
                        ridx[0] += 1

            def fin_ops(gi, graphs=None, rts=None):
                # fin matvecs per 3-graph group, rotating across PE
                # column-groups so the 512-column streams overlap 3-way
                group = graphs[gi:gi + 3]
                bb = group[0] // 3
                for kh in range(2):
                    for s in range(CHUNK // SUB):
                        for ns in range(SUB // MM):
                            cols = slice(s * SUB + ns * MM,
                                         s * SUB + (ns + 1) * MM)
                            for ci, g in enumerate(group):
                                nc.tensor.matmul(
                                    vps[32 * ci:32 * ci + 32, cols],
                                    lhsT=fwm8_sb[:, kh,
                                                 g * GPC:(g + 1) * GPC],
                                    rhs=rts[gi + ci][s][:, kh,
                                                        ns * MM:
                                                        (ns + 1) * MM],
                                    start=(bb == 0 and kh == 0),
                                    stop=(bb == last_bb[ci] and kh == 1),
                                    skip_group_check=True,
                                )

            # emit this SB's main matmuls with the PREVIOUS SB's fin
            # matvecs interleaved at the stationary switch: fin relu inputs
            # finished a whole SB ago (PE never stalls on relu), and the
            # DoubleRow LDWEIGHTS for the next mh half hides behind the
            # fin matmul stream
            relu_ops(0, range(ng))
            if pending_fin:
                pending_fin[0]()
            relu_ops(1, range(ng))
            for fin in pending_fin[1:]:
                fin()
            pending_fin = [
                (lambda gi=gi, fo=fin_ops, gr=graphs, rr=rts:
                 fo(gi, graphs=gr, rts=rr)) for gi in range(0, ng, 3)
            ]
            if k == NSB - 1:
                # warm the Exp table while the tail batches run so the
                # epilogue exp doesn't pay the load
                nc.scalar.activation(warm[0:1, 1:2], fb_sb[0:1, 0:1], AF.Exp)
        for fin in pending_fin:
            fin()

        # ---- epilogue: log_softmax over [v + fin_b | zeros] per graph ----
        # v is O(1), so no max-shift is needed: exp(v+fb) cannot overflow.
        # exp/reduce are split in column halves so ACT and DVE pipeline.
        HB = NPG // 2
        e_sb = epool.tile([96, CHUNK], f32, tag="e_sb")
        nc.scalar.activation(e_sb[:, 0:HB], vps[:, 0:HB], AF.Exp,
                             bias=fb_sb[:, 0:1])
        nc.scalar.activation(e_sb[:, HB:NPG], vps[:, HB:NPG], AF.Exp,
                             bias=fb_sb[:, 0:1])
        sh = epool.tile([96, 2], f32, tag="sh")
        nc.vector.tensor_reduce(sh[:, 0:1], e_sb[:, 0:HB], AX.X, ALU.add)
        nc.vector.tensor_reduce(sh[:, 1:2], e_sb[:, HB:NPG], AX.X, ALU.add)
        # s = sum halves + (AS - NPG) * exp(0)  (the implicit-zeros region)
        st = epool.tile([96, 1], f32, tag="st")
        nc.vector.tensor_scalar(
            out=st[:], in0=sh[:, 0:1], scalar1=sh[:, 1:2],
            scalar2=float(AS - NPG), op0=ALU.add, op1=ALU.add,
        )
        lse = epool.tile([96, 1], f32, tag="lse")
        nc.scalar.activation(lse[:], st[:], AF.Ln)
        nlse = epool.tile([96, 1], f32, tag="nlse")
        nc.vector.tensor_scalar_mul(nlse[:], lse[:], -1.0)
        bias2 = epool.tile([96, 1], f32, tag="bias2")   # fin_b - lse
        nc.vector.tensor_tensor(
            out=bias2[:], in0=fb_sb[:, 0:1], in1=lse[:], op=ALU.subtract
        )
        out_sb = epool.tile([96, AS], f32, tag="out_sb")
        # zeros region first (one DVE 2x op from SBUF), so its DMA can
        # overlap the v-region compute
        nc.vector.tensor_scalar(
            out=out_sb[:, NPG:AS], in0=e_sb[:, 0:NPG], scalar1=0.0,
            scalar2=nlse[:, 0:1], op0=ALU.mult, op1=ALU.add,
        )
        nc.scalar.activation(
            out_sb[:, 0:HB], vps[:, 0:HB], AF.Identity,
            bias=bias2[:, 0:1],
        )
        nc.vector.tensor_scalar_add(
            out_sb[:, HB:NPG], vps[:, HB:NPG], bias2[:, 0:1]
        )
        # two DMAs for all 96 partition-rows (the host picks out the 32
        # real graph rows): the zeros half is ready first and its transfer
        # overlaps the v-half compute
        nc.sync.dma_start(out_d[:, NPG:AS], out_sb[:, NPG:AS])
        nc.sync.dma_start(out_d[:, 0:NPG], out_sb[:, 0:NPG])

    nc.compile()
    return nc


def _host_reference(x, batch, lin_w, lin_b, bn_gamma, bn_beta, fin_w, fin_b,
                    batch_sz):
    h = x @ lin_w.T + lin_b
    mean = h.mean(axis=0)
    var = np.mean(np.square(h - mean), axis=0)
    h = (h - mean) / np.sqrt(var + BN_EPS) * bn_gamma + bn_beta
    h = np.maximum(h, 0.0)
    v = (h @ fin_w.T + fin_b)[:, 0]
    out = np.zeros((int(batch_sz), AS), dtype=v.dtype)
    np.add.at(out, (batch[:, 0], batch[:, 1]), v)
    m = out.max(axis=1, keepdims=True)
    lse = m + np.log(np.exp(out - m).sum(axis=1, keepdims=True))
    return (out - lse).astype(np.float32)


def _prep_fp8(x, lin_w, lin_b, bn_gamma, bn_beta):
    """Quantize x and the BN-folded weights to fp8 e4m3, computing the BN
    affine from the statistics of the quantized computation itself so the
    device's h comes out with exactly the right mean and ~exact variance."""
    import ml_dtypes

    f8 = ml_dtypes.float8_e4m3
    xq = x.astype(f8)                               # [N, C] fp8
    x8f = xq.astype(np.float32)

    # exact first/second moments of the quantized x (f64 accumulate via
    # f32 sgemm is plenty: values are O(1), N = 524288)
    S1 = x8f.sum(axis=0, dtype=np.float64)          # [C]
    G = (x8f.T @ x8f).astype(np.float64)            # [C, C]
    xbar = S1 / N
    M = G / N - np.outer(xbar, xbar)                # covariance of x8

    W = lin_w.astype(np.float64)                    # [cout, cin]
    gam = bn_gamma.astype(np.float64)
    bet = bn_beta.astype(np.float64)

    # initial fold scale from the quantized-x stats with fp32 weights
    var0 = np.einsum("ck,kl,cl->c", W, M, W, optimize=True)
    s = gam / np.sqrt(var0 + BN_EPS)                # [cout]

    # refinement: quantize, measure the quantized weights' variance,
    # rescale, requantize.  Residual scale error is O(1e-3).
    for _ in range(2):
        Wq = (W * s[:, None]).T.astype(np.float32).astype(f8)   # [cin, cout]
        Wqf = Wq.astype(np.float64)
        varq = np.einsum("kc,kl,lc->c", Wqf, M, Wqf, optimize=True)
        # multiplicative correction toward std(h_dev) = gam
        s = s * gam / np.sqrt(varq + BN_EPS)

    Wq = (W * s[:, None]).T.astype(np.float32).astype(f8)       # final
    Wqf = Wq.astype(np.float64)
    varq = np.einsum("kc,kl,lc->c", Wqf, M, Wqf, optimize=True)
    meanq = xbar @ Wqf                              # [cout], no lin_b here
    rho = np.sqrt(varq + BN_EPS)                    # device h std
    # reference: (h_ref - mu)/sigma * gam + bet ; device h ~ N(meanq, rho^2)
    # device relu input = h + bv ; choose bv = bet - (gam/rho)*meanq and
    # accept the (gam/rho - 1) ~ 1e-3 scale residual.  lin_b only shifts the
    # mean of the reference h, which BN removes, so it never appears here.
    bvec = bet - meanq * (gam / rho)

    # DoubleRow interleave: [ki, ks, m] with input channel = ks*128 + ki
    wt8 = np.ascontiguousarray(
        Wq.reshape(2, 128, C).transpose(1, 0, 2))   # [128, 2, C]
    return xq, wt8, bvec.astype(np.float32)


def kernel(**inputs):
    global _PROG, LAST_RESULTS
    x = np.asarray(inputs["x"], dtype=np.float32)
    batch = np.asarray(inputs["batch"])
    lin_w = np.asarray(inputs["lin_w"], dtype=np.float32)
    lin_b = np.asarray(inputs["lin_b"], dtype=np.float32)
    bn_gamma = np.asarray(inputs["bn_gamma"], dtype=np.float32)
    bn_beta = np.asarray(inputs["bn_beta"], dtype=np.float32)
    fin_w = np.asarray(inputs["fin_w"], dtype=np.float32)
    fin_b = np.asarray(inputs["fin_b"], dtype=np.float32)
    batch_sz = int(np.asarray(inputs["batch_sz"]))

    idx = np.arange(N, dtype=np.int64)
    b64 = batch.astype(np.int64, copy=False)
    if not (
        x.shape == (N, C)
        and batch.shape == (N, 2)
        and batch_sz == NG
        and np.array_equal(b64[:, 0], idx // NPG)
        and np.array_equal(b64[:, 1], idx % NPG)
    ):
        return _host_reference(
            x, b64, lin_w, lin_b, bn_gamma, bn_beta, fin_w, fin_b, batch_sz
        )

    import ml_dtypes
    f8 = ml_dtypes.float8_e4m3
    xq, wt8, bvec = _prep_fp8(x, lin_w, lin_b, bn_gamma, bn_beta)
    # masked fin_w stationaries: fwm8[k, kh, g*32+j] = fw[kh*128+k]*(j==g//3)
    fwm8 = np.zeros((128, 2, GPC * GPC), dtype=f8)
    fwf = fin_w[0].astype(np.float32)
    for g in range(GPC):
        for kh in range(2):
            fwm8[:, kh, g * GPC + g // 3] = fwf[
                kh * 128:(kh + 1) * 128].astype(f8)
    bvv = np.ascontiguousarray(bvec[:, None], dtype=np.float32)
    fbv = np.full((96, 1), float(fin_b[0]), dtype=np.float32)

    import time as _time
    _t = _time.time()
    if _PROG is None:
        _PROG = _build_program()
    nc = _PROG
    print(f"[kernel] build done {_time.time()-_t:.1f}s", flush=True)

    in_maps = []
    for i in range(NCORES):
        xs = xq[i * NLOC:(i + 1) * NLOC]            # [NLOC, C] fp8
        x8 = np.ascontiguousarray(
            xs.T.reshape(2, 128, NLOC).transpose(1, 0, 2))  # [128, 2, NLOC]
        in_maps.append({"x8": x8, "wt8": wt8, "fwm8": fwm8, "bv": bvv,
                        "fb": fbv})

    from concourse.bass_utils import run_bass_kernel_spmd

    _t = _time.time()
    res = run_bass_kernel_spmd(
        nc, in_maps, list(range(NCORES)), trace=TRACE
    )
    print(f"[kernel] run done {_time.time()-_t:.1f}s", flush=True)
    LAST_RESULTS = res
    rowsel = np.array([32 * (g % 3) + g // 3 for g in range(GPC)],
                      dtype=np.int64)
    return np.concatenate(
        [res.results[i]["out"][rowsel] for i in range(NCORES)], axis=0
    )


# revision 23
# speedup vs baseline: 1.0101x; 1.0101x over previous
"""Trainium2 Bass kernel for nn_PolicyHead_1Trunk (scatter_memory).

Computation (reference):
    h = x @ lin_w.T + lin_b                  # [N, 256]
    h = batchnorm(h) (training stats over N) ; relu
    v = (h @ fin_w.T + fin_b)[:, 0]          # [N]
    out = scatter_add(v, batch) -> [256, 4096]; log_softmax rows

Strategy:
  * batch is the identity COO pattern [i // 2048, i % 2048] (verified on
    host; falls back to a numpy path if not).
  * BN batch statistics are folded into a per-channel affine on the host.
    To make fp8 quantization error *systematically* cancel, the affine is
    computed from the statistics of the QUANTIZED computation (x8^T x8 with
    the quantized weights, scale-refinement iterations), so the device's
    h has exactly the right mean and (to ~0.1%) the right variance; only
    per-node quantization noise survives, which the 2e-2 gate forgives.
  * x is shipped as fp8 e4m3 in DoubleRow interleave [128, 2, NLOC]
    (ki, ks, n) = x[n, ks*128+ki]: 16 MB/core of HBM traffic instead of
    64 MB.  The main matmul is a single K=256 DoubleRow fp8 matmul per
    512-node slice (2x PE throughput vs bf16).
  * Data-parallel over graphs: core i owns rows [i*65536, (i+1)*65536)
    (32 whole graphs).
  * Per core: stream x8 super-batches (6 graphs = 3 MB per DMA), DoubleRow
    matmul with the BN-folded fp8 weights (mh-outer so the stationary only
    changes twice per super-batch), fused bias+relu (ACT/DVE 13:11 split)
    into fp8 rt tiles, then a plain-fp8 fin matvec (runs at bf16 rate) via
    masked stationaries into a persistent PSUM tile [96, 2048] — graph g
    lives at partition 32*(g%3) + g//3, so fin matvecs of 3 consecutive
    graphs run concurrently on PE column-groups 0/32/64 — and finally a
    shift-free log-softmax epilogue over [32, 4096] rows including the
    2048 implicit zero entries (v is O(1) so exp cannot overflow).
"""

import os
import sys

import numpy as np

for _p in ("/opt/trn_rl_repo", "/root/.axon_site/_ro/trn_rl_repo"):
    if os.path.isdir(_p) and _p not in sys.path:
        sys.path.insert(0, _p)

C = 256           # channels
NPG = 2048        # nodes per graph
NG = 256          # graphs
N = NG * NPG      # 524288 nodes
AS = 4096         # action size
NCORES = 8
GPC = NG // NCORES          # 32 graphs per core
NLOC = GPC * NPG            # 65536 rows per core
BN_EPS = 1e-5

_PROG = None      # cached — compile once per process
TRACE = False     # test.py can flip this for ntff profiling
LAST_RESULTS = None


def _build_program():
    import concourse.bass as bass
    import concourse.tile as tile
    from concourse import bacc, mybir
    from contextlib import ExitStack

    f32 = mybir.dt.float32
    f8 = mybir.dt.float8e4
    AF = mybir.ActivationFunctionType
    ALU = mybir.AluOpType
    AX = mybir.AxisListType
    DR = mybir.MatmulPerfMode.DoubleRow

    nc = bacc.Bacc(
        "TRN2", target_bir_lowering=False, debug=False, enable_asserts=False
    )

    # x in DoubleRow interleave: x8[ki, ks, n] = x[n, ks*128+ki] (fp8 e4m3)
    x8 = nc.dram_tensor("x8", [128, 2, NLOC], f8, kind="ExternalInput").ap()
    # weights in DoubleRow interleave: wt8[ki, ks, m] = W_dev[ks*128+ki, m]
    wt8 = nc.dram_tensor("wt8", [128, 2, C], f8, kind="ExternalInput").ap()
    # fwm8[k, kh, g*32 + j] = fin_w[kh*128+k] * (j == g//3): masked copies
    # of the final projection.  An M=32 matmul with this stationary writes
    # v into one PSUM partition and +0 elsewhere, so PSUM accumulation
    # doubles as the per-graph scatter.
    fwm8 = nc.dram_tensor("fwm8", [128, 2, GPC * GPC], f8,
                          kind="ExternalInput").ap()
    bv = nc.dram_tensor("bv", [C, 1], f32, kind="ExternalInput").ap()
    fb = nc.dram_tensor("fb", [96, 1], f32, kind="ExternalInput").ap()
    out_d = nc.dram_tensor("out", [96, AS], f32, kind="ExternalOutput").ap()

    CHUNK = NPG           # 2048 rows per chunk == one graph
    NCH = NLOC // CHUNK   # 32 chunks
    SB = 6                # graphs per super-batch (one x DMA, 2 LDWEIGHTS)
    SUB = 1024            # columns per PSUM tile
    MM = 512              # output columns per matmul

    with tile.TileContext(nc) as tc, ExitStack() as ctx:
        consts = ctx.enter_context(tc.tile_pool(name="consts", bufs=1))
        xpool = ctx.enter_context(tc.tile_pool(name="x", bufs=3))
        rpool = ctx.enter_context(tc.tile_pool(name="relu", bufs=2))
        hpool = ctx.enter_context(tc.tile_pool(name="h", bufs=2, space="PSUM"))
        vpool = ctx.enter_context(tc.tile_pool(name="v", bufs=1, space="PSUM"))
        epool = ctx.enter_context(tc.tile_pool(name="epi", bufs=1))

        # ---- constants into SBUF via the scalar HWDGE queue (fast setup;
        # the sync queue is kept free to start streaming x immediately) ----
        wt8_sb = consts.tile([128, 2, C], f8, tag="wt8")
        nc.scalar.dma_start(wt8_sb[:], wt8[:, :, :])
        bv_sb = []   # [128, 1] per output-channel half
        for mh in range(2):
            t = consts.tile([128, 1], f32, tag=f"bv{mh}")
            nc.scalar.dma_start(t[:], bv[mh * 128:(mh + 1) * 128, :])
            bv_sb.append(t)
        fwm8_sb = consts.tile([128, 2, GPC * GPC], f8, tag="fwm8")
        nc.scalar.dma_start(fwm8_sb[:], fwm8[:, :, :])
        fb_sb = consts.tile([96, 1], f32, tag="fb")
        nc.scalar.dma_start(fb_sb[:], fb[:, :])

        # warm the ACT Relu table so the first bias+relu doesn't stall the
        # PE->relu pipeline behind a table load
        warm = epool.tile([1, 2], f32, tag="warm")
        nc.scalar.activation(warm[0:1, 0:1], fb_sb[0:1, 0:1], AF.Relu)

        # persistent PSUM accumulator for v: graph g lives at partition
        # 32*(g%3) + g//3 — g%3 selects the PE column-group so that the
        # fin matvecs of 3 consecutive graphs run concurrently on
        # column-groups 0/32/64 of the systolic array.
        vps = vpool.tile([96, CHUNK], f32, tag="vps")
        NSB = (NCH + SB - 1) // SB              # 6 super-batches
        last_bb = [(NCH - 1 - ci) // 3 for ci in range(3)]
        pending_fin = []

        for k in range(NSB):
            graphs = [g for g in range(k * SB, min((k + 1) * SB, NCH))]
            ng = len(graphs)
            # one DMA for the whole super-batch's x (6 graphs = 3 MB);
            # super-batch 0 is split per-graph so the first matmul can
            # start sooner
            xb = xpool.tile([128, 2, SB * CHUNK], f8, tag="xb")
            c0 = graphs[0] * CHUNK
            if k == 0:
                nc.sync.dma_start(
                    xb[:, :, 0:MM], x8[:, :, 0:MM]
                )
                nc.sync.dma_start(
                    xb[:, :, MM:CHUNK], x8[:, :, MM:CHUNK]
                )
                for ci in range(1, ng):
                    nc.sync.dma_start(
                        xb[:, :, ci * CHUNK:(ci + 1) * CHUNK],
                        x8[:, :, c0 + ci * CHUNK:c0 + (ci + 1) * CHUNK],
                    )
            else:
                nc.sync.dma_start(
                    xb[:, :, 0:ng * CHUNK], x8[:, :, c0:c0 + ng * CHUNK]
                )
            # rt tiles: [128, 2, SUB] fp8, mh halves stacked — one tile per
            # (ci, s); the fin matvec reads each half as a [128, 512] slice
            rts = [[rpool.tile([128, 2, SUB], f8, tag=f"r{ci}_{s}",
                               name=f"rt{ci}{s}")
                    for s in range(CHUNK // SUB)] for ci in range(ng)]
            ridx = [0]

            def relu_ops(mh, cis):
                for ci in cis:
                    for s in range(CHUNK // SUB):
                        hps = hpool.tile([128, SUB], f32, tag="hps",
                                         name="hps")
                        for ns in range(SUB // MM):
                            col = ci * CHUNK + s * SUB + ns * MM
                            nc.tensor.matmul(
                                hps[:, ns * MM:(ns + 1) * MM],
                                lhsT=wt8_sb[:, :, mh * 128:(mh + 1) * 128],
                                rhs=xb[:, :, col:col + MM],
                                start=True,
                                stop=True,
                                perf_mode=DR,
                            )
                        rt = rts[ci][s][:, mh, :]
                        if ridx[0] % 2 == 0 or (ridx[0] == 23 and k % 2 == 0):
                            nc.scalar.activation(
                                rt, hps[:], AF.Relu,
                                bias=bv_sb[mh][:, 0:1],
                            )
                        else:
                            nc.vector.tensor_scalar(
                                out=rt, in0=hps[:],
                                scalar1=bv_sb[mh][:, 0:1], scalar2=0.0,
                                op0=ALU.add, op1=ALU.max,
                            )
                        ridx[0] += 1

            def fin_ops(gi, graphs=None, rts=None):
                # fin matvecs per 3-graph group, rotating across PE
                # column-groups so the 512-column streams overlap 3-way
                group = graphs[gi:gi + 3]
                bb = group[0] // 3
                for kh in range(2):
                    for s in range(CHUNK // SUB):
                        for ns in range(SUB // MM):
                            cols = slice(s * SUB + ns * MM,
                                         s * SUB + (ns + 1) * MM)
                            for ci, g in enumerate(group):
                                nc.tensor.matmul(
                                    vps[32 * ci:32 * ci + 32, cols],
                                    lhsT=fwm8_sb[:, kh,
                                                 g * GPC:(g + 1) * GPC],
                                    rhs=rts[gi + ci][s][:, kh,
                                                        ns * MM:
                                                        (ns + 1) * MM],
                                    start=(bb == 0 and kh == 0),
                                    stop=(bb == last_bb[ci] and kh == 1),
                                    skip_group_check=True,
                                )

            # emit this SB's main matmuls with the PREVIOUS SB's fin
            # matvecs interleaved at the stationary switch: fin relu inputs
            # finished a whole SB ago (PE never stalls on relu), and the
            # DoubleRow LDWEIGHTS for the next mh half hides behind the
            # fin matmul stream
            relu_ops(0, range(ng))
            if pending_fin:
                pending_fin[0]()
            relu_ops(1, range(ng))
            for fin in pending_fin[1:]:
                fin()
            pending_fin = [
                (lambda gi=gi, fo=fin_ops, gr=graphs, rr=rts:
                 fo(gi, graphs=gr, rts=rr)) for gi in range(0, ng, 3)
            ]
            if k == NSB - 1:
                # warm the Exp table while the tail batches run so the
                # epilogue exp doesn't pay the load
                nc.scalar.activation(warm[0:1, 1:2], fb_sb[0:1, 0:1], AF.Exp)
        for fin in pending_fin:
            fin()

        # ---- epilogue: log_softmax over [v + fin_b | zeros] per graph ----
        # v is O(1), so no max-shift is needed: exp(v+fb) cannot overflow.
        # exp/reduce are split in column halves so ACT and DVE pipeline.
        HB = NPG // 2
        e_sb = epool.tile([96, CHUNK], f32, tag="e_sb")
        nc.scalar.activation(e_sb[:, 0:HB], vps[:, 0:HB], AF.Exp,
                             bias=fb_sb[:, 0:1])
        nc.scalar.activation(e_sb[:, HB:NPG], vps[:, HB:NPG], AF.Exp,
                             bias=fb_sb[:, 0:1])
        sh = epool.tile([96, 2], f32, tag="sh")
        nc.vector.tensor_reduce(sh[:, 0:1], e_sb[:, 0:HB], AX.X, ALU.add)
        nc.vector.tensor_reduce(sh[:, 1:2], e_sb[:, HB:NPG], AX.X, ALU.add)
        # s = sum halves + (AS - NPG) * exp(0)  (the implicit-zeros region)
        st = epool.tile([96, 1], f32, tag="st")
        nc.vector.tensor_scalar(
            out=st[:], in0=sh[:, 0:1], scalar1=sh[:, 1:2],
            scalar2=float(AS - NPG), op0=ALU.add, op1=ALU.add,
        )
        lse = epool.tile([96, 1], f32, tag="lse")
        nc.scalar.activation(lse[:], st[:], AF.Ln)
        nlse = epool.tile([96, 1], f32, tag="nlse")
        nc.vector.tensor_scalar_mul(nlse[:], lse[:], -1.0)
        bias2 = epool.tile([96, 1], f32, tag="bias2")   # fin_b - lse
        nc.vector.tensor_tensor(
            out=bias2[:], in0=fb_sb[:, 0:1], in1=lse[:], op=ALU.subtract
        )
        out_sb = epool.tile([96, AS], f32, tag="out_sb")
        # zeros region first (one DVE 2x op from SBUF), so its DMA can
        # overlap the v-region compute
        nc.vector.tensor_scalar(
            out=out_sb[:, NPG:AS], in0=e_sb[:, 0:NPG], scalar1=0.0,
            scalar2=nlse[:, 0:1], op0=ALU.mult, op1=ALU.add,
        )
        nc.scalar.activation(
            out_sb[:, 0:HB], vps[:, 0:HB], AF.Identity,
            bias=bias2[:, 0:1],
        )
        nc.vector.tensor_scalar_add(
            out_sb[:, HB:NPG], vps[:, HB:NPG], bias2[:, 0:1]
        )
        # two DMAs for all 96 partition-rows (the host picks out the 32
        # real graph rows): the zeros half is ready first and its transfer
        # overlaps the v-half compute
        nc.gpsimd.dma_start(out_d[:, NPG:AS], out_sb[:, NPG:AS])
        nc.sync.dma_start(out_d[:, 0:NPG], out_sb[:, 0:NPG])

    nc.compile()
    return nc


def _host_reference(x, batch, lin_w, lin_b, bn_gamma, bn_beta, fin_w, fin_b,
                    batch_sz):
    h = x @ lin_w.T + lin_b
    mean = h.mean(axis=0)
    var = np.mean(np.square(h - mean), axis=0)
    h = (h - mean) / np.sqrt(var + BN_EPS) * bn_gamma + bn_beta
    h = np.maximum(h, 0.0)
    v = (h @ fin_w.T + fin_b)[:, 0]
    out = np.zeros((int(batch_sz), AS), dtype=v.dtype)
    np.add.at(out, (batch[:, 0], batch[:, 1]), v)
    m = out.max(axis=1, keepdims=True)
    lse = m + np.log(np.exp(out - m).sum(axis=1, keepdims=True))
    return (out - lse).astype(np.float32)


def _prep_fp8(x, lin_w, lin_b, bn_gamma, bn_beta):
    """Quantize x and the BN-folded weights to fp8 e4m3, computing the BN
    affine from the statistics of the quantized computation itself so the
    device's h comes out with exactly the right mean and ~exact variance."""
    import ml_dtypes

    f8 = ml_dtypes.float8_e4m3
    xq = x.astype(f8)                               # [N, C] fp8
    x8f = xq.astype(np.float32)

    # exact first/second moments of the quantized x (f64 accumulate via
    # f32 sgemm is plenty: values are O(1), N = 524288)
    S1 = x8f.sum(axis=0, dtype=np.float64)          # [C]
    G = (x8f.T @ x8f).astype(np.float64)            # [C, C]
    xbar = S1 / N
    M = G / N - np.outer(xbar, xbar)                # covariance of x8

    W = lin_w.astype(np.float64)                    # [cout, cin]
    gam = bn_gamma.astype(np.float64)
    bet = bn_beta.astype(np.float64)

    # initial fold scale from the quantized-x stats with fp32 weights
    var0 = np.einsum("ck,kl,cl->c", W, M, W, optimize=True)
    s = gam / np.sqrt(var0 + BN_EPS)                # [cout]

    # refinement: quantize, measure the quantized weights' variance,
    # rescale, requantize.  Residual scale error is O(1e-3).
    for _ in range(2):
        Wq = (W * s[:, None]).T.astype(np.float32).astype(f8)   # [cin, cout]
        Wqf = Wq.astype(np.float64)
        varq = np.einsum("kc,kl,lc->c", Wqf, M, Wqf, optimize=True)
        # multiplicative correction toward std(h_dev) = gam
        s = s * gam / np.sqrt(varq + BN_EPS)

    Wq = (W * s[:, None]).T.astype(np.float32).astype(f8)       # final
    Wqf = Wq.astype(np.float64)
    varq = np.einsum("kc,kl,lc->c", Wqf, M, Wqf, optimize=True)
    meanq = xbar @ Wqf                              # [cout], no lin_b here
    rho = np.sqrt(varq + BN_EPS)                    # device h std
    # reference: (h_ref - mu)/sigma * gam + bet ; device h ~ N(meanq, rho^2)
    # device relu input = h + bv ; choose bv = bet - (gam/rho)*meanq and
    # accept the (gam/rho - 1) ~ 1e-3 scale residual.  lin_b only shifts the
    # mean of the reference h, which BN removes, so it never appears here.
    bvec = bet - meanq * (gam / rho)

    # DoubleRow interleave: [ki, ks, m] with input channel = ks*128 + ki
    wt8 = np.ascontiguousarray(
        Wq.reshape(2, 128, C).transpose(1, 0, 2))   # [128, 2, C]
    return xq, wt8, bvec.astype(np.float32)


def kernel(**inputs):
    global _PROG, LAST_RESULTS
    x = np.asarray(inputs["x"], dtype=np.float32)
    batch = np.asarray(inputs["batch"])
    lin_w = np.asarray(inputs["lin_w"], dtype=np.float32)
    lin_b = np.asarray(inputs["lin_b"], dtype=np.float32)
    bn_gamma = np.asarray(inputs["bn_gamma"], dtype=np.float32)
    bn_beta = np.asarray(inputs["bn_beta"], dtype=np.float32)
    fin_w = np.asarray(inputs["fin_w"], dtype=np.float32)
    fin_b = np.asarray(inputs["fin_b"], dtype=np.float32)
    batch_sz = int(np.asarray(inputs["batch_sz"]))

    idx = np.arange(N, dtype=np.int64)
    b64 = batch.astype(np.int64, copy=False)
    if not (
        x.shape == (N, C)
        and batch.shape == (N, 2)
        and batch_sz == NG
        and np.array_equal(b64[:, 0], idx // NPG)
        and np.array_equal(b64[:, 1], idx % NPG)
    ):
        return _host_reference(
            x, b64, lin_w, lin_b, bn_gamma, bn_beta, fin_w, fin_b, batch_sz
        )

    import ml_dtypes
    f8 = ml_dtypes.float8_e4m3
    xq, wt8, bvec = _prep_fp8(x, lin_w, lin_b, bn_gamma, bn_beta)
    # masked fin_w stationaries: fwm8[k, kh, g*32+j] = fw[kh*128+k]*(j==g//3)
    fwm8 = np.zeros((128, 2, GPC * GPC), dtype=f8)
    fwf = fin_w[0].astype(np.float32)
    for g in range(GPC):
        for kh in range(2):
            fwm8[:, kh, g * GPC + g // 3] = fwf[
                kh * 128:(kh + 1) * 128].astype(f8)
    bvv = np.ascontiguousarray(bvec[:, None], dtype=np.float32)
    fbv = np.full((96, 1), float(fin_b[0]), dtype=np.float32)

    import time as _time
    _t = _time.time()
    if _PROG is None:
        _PROG = _build_program()
    nc = _PROG
    print(f"[kernel] build done {_time.time()-_t:.1f}s", flush=True)

    in_maps = []
    for i in range(NCORES):
        xs = xq[i * NLOC:(i + 1) * NLOC]            # [NLOC, C] fp8
        x8 = np.ascontiguousarray(
            xs.T.reshape(2, 128, NLOC).transpose(1, 0, 2))  # [128, 2, NLOC]
        in_maps.append({"x8": x8, "wt8": wt8, "fwm8": fwm8, "bv": bvv,
                        "fb": fbv})

    from concourse.bass_utils import run_bass_kernel_spmd

    _t = _time.time()
    res = run_bass_kernel_spmd(
        nc, in_maps, list(range(NCORES)), trace=TRACE
    )
    print(f"[kernel] run done {_time.time()-_t:.1f}s", flush=True)
    LAST_RESULTS = res
    rowsel = np.array([32 * (g % 3) + g // 3 for g in range(GPC)],
                      dtype=np.int64)
    return np.concatenate(
        [res.results[i]["out"][rowsel] for i in range(NCORES)], axis=0
    )
